# revision 1
# baseline (speedup 1.0000x reference)
"""Trainium2 Bass kernel for nn_DRL4SSP (pointer-network greedy decode).

Strategy: pure data-parallel over batch B=64 across 8 NeuronCores (8 items
per core). Inside each core the 127 sequential decode steps run fully
on-chip: encoders/bases are computed once in a prologue; the per-step
recurrence (GRU + two pointer-attention stages + greedy argmax) is executed
with all state resident in SBUF/PSUM. Two pipeline groups of 4 batch items
interleave to hide the cross-engine dependency chain.

Key layout choices (per core, b = local batch 0..7, s = position 0..127):
  base1P/base2P   [128(h), 1024(b-major, s)]   loop-invariant bias tensors
  W2SHT/WdecST    [128(s), 1024(b-major, h)]   per-item folded weights
  attn/softmax    [128(s), nb] transposed form; softmax sums are
                  partition-replicated via a ones-matrix matmul so stage 1
                  needs no partition reshapes at all.
  logits          transposed->block via one PE transpose; argmax via DVE
                  max/max_index; log-prob denominators deferred to a single
                  post-loop pass (keeps the hot loop on one ACT table set).
All compute is fp32: bf16 was measured to flip 63/64 tours and float32r
(TF32-class, ~2e-4 rounding) flipped 18/64, so the broadcast-adds run as
DVE tensor-tensor adds with 0-stride APs instead of PE identity-matmuls.
"""
import sys
import numpy as np

for _p in ("/opt/trn_rl_repo",):
    if _p not in sys.path:
        sys.path.insert(0, _p)

B, SS, DS, H, S = 64, 8, 4, 128, 128
NCORES = 8
BL = B // NCORES          # batch items per core = 8
NG = 2                    # pipeline groups per core
GB = BL // NG             # batch items per group = 4
NSTEP = S - 1             # 127
NEG = -1e30


def _build_nc(n_steps=NSTEP, bench_loop=1):
    from contextlib import ExitStack
    import concourse.bass as bass
    import concourse.tile as tile
    from concourse import bacc, mybir

    f32 = mybir.dt.float32
    f32r = mybir.dt.float32r
    u32 = mybir.dt.uint32
    AF = mybir.ActivationFunctionType
    OP = mybir.AluOpType

    nc = bacc.Bacc("TRN2", target_bir_lowering=False, debug=False,
                   enable_asserts=False)

    # ---- DRAM I/O ----
    din = {}
    def dram_in(name, shape):
        din[name] = nc.dram_tensor(name, shape, f32, kind="ExternalInput").ap()
    dram_in("staticT8", [SS, BL * S])      # [i, (b,s)]
    dram_in("dynT4", [DS, BL * S])
    dram_in("penT0", [S, BL])              # penalty, transposed [s, b]
    for nm, shp in [("WsT", [SS, H]), ("WdT", [DS, H]),
                    ("ww1sT", [H, H]), ("ww1dT", [H, H]), ("w1hT", [H, H]),
                    ("ww2sT", [H, H]), ("ww2dT", [H, H]), ("w2dT", [H, H]),
                    ("WdecT", [SS, H]),
                    ("WihT_r", [H, H]), ("WihT_z", [H, H]), ("WihT_n", [H, H]),
                    ("WhhT_r", [H, H]), ("WhhT_z", [H, H]), ("WhhT_nh", [H, H]),
                    ("vv1c", [H, 1]), ("vv2c", [H, 1]),
                    ("I128", [H, H]), ("ones128", [H, H])]:
        dram_in(nm, shp)
    nchunk_o = (GB * n_steps + S - 1) // S
    out_idx = nc.dram_tensor("out_idx_raw", [BL, n_steps], u32,
                             kind="ExternalOutput").ap()
    out_logp = nc.dram_tensor("out_logp_raw", [H, NG * nchunk_o], f32,
                              kind="ExternalOutput").ap()

    r = lambda ap: ap.bitcast(f32r)

    with ExitStack() as ctx:
        tc = ctx.enter_context(tile.TileContext(nc))
        cpool = ctx.enter_context(tc.tile_pool(name="consts", bufs=1))
        state = ctx.enter_context(tc.tile_pool(name="state", bufs=1))
        work = ctx.enter_context(tc.tile_pool(name="work", bufs=3))

        if bench_loop > 1:
            loop_cm = tc.For_i(0, bench_loop, 1)
        else:
            loop_cm = None
        from contextlib import nullcontext
        with (loop_cm if loop_cm is not None else nullcontext()):
            # ---- load constants to SBUF ----
            # Every const is copied once through DVE so that downstream matmuls
            # depend on a single engine semaphore (fp32 self-loading matmuls
            # tolerate only one sync wait).
            cs = {}
            for nm, ap in din.items():
                raw = cpool.tile(list(ap.shape), f32, tag=f"r_{nm}", name=f"r_{nm}")
                nc.sync.dma_start(raw[:], ap[:])
                t = cpool.tile(list(ap.shape), f32, tag=nm, name=f"c_{nm}")
                nc.vector.tensor_copy(out=t[:], in_=raw[:])
                cs[nm] = t

            # ---- persistent state ----
            base1P = state.tile([H, BL * S], f32, tag="base1P")
            base2P = state.tile([H, BL * S], f32, tag="base2P")
            W2SHT = state.tile([S, BL * H], f32, tag="W2SHT")
            WdecST = state.tile([S, BL * H], f32, tag="WdecST")
            hT = [state.tile([H, GB], f32, tag=f"hT{g}", name=f"hT_{g}")
                  for g in range(NG)]
            dec_hT = [state.tile([H, GB], f32, tag=f"dhT{g}", name=f"dhT_{g}")
                      for g in range(NG)]
            penaltyT = [state.tile([S, GB], f32, tag=f"penT{g}", name=f"penT_{g}")
                        for g in range(NG)]
            # per-group logit store in transposed [s, (t,b)] form + ptr store
            logbT = [state.tile([S, GB * n_steps], f32, tag=f"logbT{g}",
                                name=f"logbT_{g}") for g in range(NG)]
            ptrb = [state.tile([GB, n_steps], u32, tag=f"ptrb{g}",
                               name=f"ptrb_{g}") for g in range(NG)]
            shS = state.tile([H, BL * S], f32, tag="shS")       # static_h
            dhS = state.tile([H, BL * S], f32, tag="dhS")       # dynamic_h

            for g in range(NG):
                nc.vector.memset(hT[g][:], 0.0)
                nc.vector.memset(dec_hT[g][:], 0.0)
                nc.vector.memset(logbT[g][:], 0.0)
                nc.vector.tensor_copy(out=penaltyT[g][:],
                                      in_=cs["penT0"][:, g * GB:(g + 1) * GB])

            # ---- prologue: encoders, bases, folded weights ----
            with tc.tile_pool(name="pro_ps", bufs=2, space="PSUM") as pps:
                def big_mm_to(dst, terms):
                    # dst[:, h0:h0+512] accumulated from [(lhsT, rhs)] fp32r mms
                    for half in range(2):
                        sl = slice(half * 512, half * 512 + 512)
                        pt = pps.tile([H, 512], f32, tag="pro")
                        for i, (lhsT, rhs) in enumerate(terms):
                            nc.tensor.matmul(pt[:], lhsT, rhs[:, sl],
                                             start=(i == 0),
                                             stop=(i == len(terms) - 1))
                        nc.vector.tensor_copy(out=dst[:, sl], in_=pt[:])

                big_mm_to(shS, [(cs["WsT"][:], cs["staticT8"][:])])
                big_mm_to(dhS, [(cs["WdT"][:], cs["dynT4"][:])])
                big_mm_to(base1P, [(cs["ww1sT"][:], shS[:]),
                                   (cs["ww1dT"][:], dhS[:])])
                big_mm_to(base2P, [(cs["ww2sT"][:], shS[:]),
                                   (cs["ww2dT"][:], dhS[:])])

                # W2SH = w2d @ static_h, then per-item transpose to [s, (b,h)]
                w2a = state.tile([H, BL * S], f32, tag="w2a")
                big_mm_to(w2a, [(cs["w2dT"][:], shS[:])])
                wda = state.tile([H, BL * S], f32, tag="wda")
                big_mm_to(wda, [(cs["WdecT"][:], cs["staticT8"][:])])
                for b in range(BL):
                    sl = slice(b * S, b * S + S)
                    pt = pps.tile([H, S], f32, tag="protr")
                    nc.tensor.transpose(pt[:], w2a[:, sl], cs["I128"][:])
                    nc.vector.tensor_copy(out=W2SHT[:, sl], in_=pt[:])
                    pt2 = pps.tile([H, S], f32, tag="protr")
                    nc.tensor.transpose(pt2[:], wda[:, sl], cs["I128"][:])
                    nc.vector.tensor_copy(out=WdecST[:, sl], in_=pt2[:])

            # ---- main-loop PSUM pools (per group) ----
            psA = [ctx.enter_context(
                tc.tile_pool(name=f"Ag{g}", bufs=1, space="PSUM")) for g in range(NG)]
            psB = [ctx.enter_context(
                tc.tile_pool(name=f"Bg{g}", bufs=1, space="PSUM")) for g in range(NG)]

            # bankA: gates r|z (0:8), nacc (8:12), hn2 (12:16), A1T (16:20),
            #        S1rep (20:24), DH (24:28), U1 (28:32)
            bkA = [psA[g].tile([H, 512], f32, tag="bka", name=f"bkA_{g}") for g in range(NG)]
            # bankB: U2 (0:4), A2T (4:8), OHT (8:12), Lblk [0:4, 16:144]
            bkB = [psB[g].tile([H, 512], f32, tag="bkb", name=f"bkB_{g}") for g in range(NG)]

            AFt, AFe = AF.Tanh, AF.Exp

            def step(t, g):
                gc = slice(g * GB, g * GB + GB)          # group batch cols
                gs = slice(g * GB * S, (g + 1) * GB * S)  # group (b,s) cols
                gh = slice(g * GB * H, (g + 1) * GB * H)  # group (b,h) cols
                ga, gb_ = bkA[g], bkB[g]
                G_r, G_z = ga[:, 0:4], ga[:, 4:8]
                G_rz, G_n, G_h2 = ga[:, 0:8], ga[:, 8:12], ga[:, 12:16]
                A1T, S1rep, DH = ga[:, 16:20], ga[:, 20:24], ga[:, 24:28]
                U1 = ga[:, 28:32]
                U2, A2T, OHT = gb_[:, 0:4], gb_[:, 4:8], gb_[:, 8:12]
                Lblk = gb_[0:GB, 16:144]
                dh_g, h_g = dec_hT[g][:], hT[g][:]

                # ---- GRU ----
                nc.tensor.matmul(G_h2, cs["WhhT_nh"][:], h_g, start=True, stop=True)
                nc.tensor.matmul(G_r, cs["WihT_r"][:], dh_g, start=True, stop=False)
                nc.tensor.matmul(G_r, cs["WhhT_r"][:], h_g, start=False, stop=True)
                nc.tensor.matmul(G_z, cs["WihT_z"][:], dh_g, start=True, stop=False)
                nc.tensor.matmul(G_z, cs["WhhT_z"][:], h_g, start=False, stop=True)
                nc.tensor.matmul(G_n, cs["WihT_n"][:], dh_g, start=True, stop=True)
                trz = work.tile([H, 2 * GB], f32, tag=f"trz{g}")
                nc.scalar.activation(trz[:], G_rz, AFt, scale=0.5)
                q2 = work.tile([H, GB], f32, tag=f"q2{g}")
                nc.vector.tensor_scalar(out=q2[:], in0=trz[:, 0:GB],
                                        scalar1=1.0, scalar2=None, op0=OP.add)
                q = work.tile([H, GB], f32, tag=f"q{g}")
                nc.vector.tensor_tensor(out=q[:], in0=q2[:], in1=G_h2,
                                        op=OP.mult)
                nin = work.tile([H, GB], f32, tag=f"nin{g}")
                nc.vector.tensor_tensor(out=nin[:], in0=q[:], in1=G_n, op=OP.add)
                tn = work.tile([H, GB], f32, tag=f"tn{g}")
                nc.scalar.activation(tn[:], nin[:], AFt)
                z2 = work.tile([H, GB], f32, tag=f"z2{g}")
                nc.vector.tensor_scalar(out=z2[:], in0=trz[:, GB:2 * GB],
                                        scalar1=0.5, scalar2=0.5,
                                        op0=OP.mult, op1=OP.add)
                v = work.tile([H, GB], f32, tag=f"v{g}")
                nc.vector.tensor_tensor(out=v[:], in0=h_g, in1=tn[:],
                                        op=OP.subtract)
                w_ = work.tile([H, GB], f32, tag=f"w{g}")
                nc.vector.tensor_tensor(out=w_[:], in0=z2[:], in1=v[:], op=OP.mult)
                nc.vector.tensor_tensor(out=h_g, in0=tn[:], in1=w_[:], op=OP.add)

                # ---- stage 1: t1 = tanh(base1 + u1), u1 = w1h @ h ----
                nc.tensor.matmul(U1, cs["w1hT"][:], h_g, start=True, stop=True)
                t1pre = work.tile([H, GB * S], f32, tag=f"t1p{g}")
                for cb in range(2):
                    cw = slice(cb * 256, cb * 256 + 256)
                    gsc = slice(g * GB * S + cb * 256, g * GB * S + cb * 256 + 256)
                    nc.vector.tensor_tensor(
                        out=t1pre[:, cw].rearrange("p (b s) -> p b s", b=2),
                        in0=base1P[:, gsc].rearrange("p (b s) -> p b s", b=2),
                        in1=U1[:, 2 * cb:2 * cb + 2, None]
                            .broadcast_to((H, 2, S)), op=OP.add)
                t1S = work.tile([H, GB * S], f32, tag=f"t1S{g}")
                nc.scalar.activation(t1S[:, 0:256], t1pre[:, 0:256], AFt)
                nc.scalar.activation(t1S[:, 256:512], t1pre[:, 256:512], AFt)
                for bl in range(GB):
                    nc.tensor.matmul(A1T[:, bl:bl + 1],
                                     t1S[:, bl * S:(bl + 1) * S], cs["vv1c"][:],
                                     start=True, stop=True)
                e1T = work.tile([S, GB], f32, tag=f"e1T{g}")
                nc.scalar.activation(e1T[:], A1T, AFe)   # softmax1 w/o max-sub
                nc.tensor.matmul(S1rep, cs["ones128"][:], e1T[:],
                                 start=True, stop=True)
                r1 = work.tile([S, GB], f32, tag=f"r1{g}")
                nc.vector.reciprocal(r1[:], S1rep)
                e1sT = work.tile([S, GB], f32, tag=f"e1sT{g}")
                nc.vector.tensor_tensor(out=e1sT[:], in0=e1T[:], in1=r1[:],
                                        op=OP.mult)

                # ---- stage 2: t2 = tanh(base2 + u2), u2 = W2SH @ softmax1 ----
                for bl in range(GB):
                    b = g * GB + bl
                    nc.tensor.matmul(U2[:, bl:bl + 1],
                                     W2SHT[:, b * H:(b + 1) * H],
                                     e1sT[:, bl:bl + 1], start=True, stop=True)
                u2S = work.tile([H, GB], f32, tag=f"u2S{g}")
                nc.vector.tensor_copy(out=u2S[:], in_=U2)
                t2pre = work.tile([H, GB * S], f32, tag=f"t2p{g}")
                for cb in range(2):
                    cw = slice(cb * 256, cb * 256 + 256)
                    gsc = slice(g * GB * S + cb * 256, g * GB * S + cb * 256 + 256)
                    nc.vector.tensor_tensor(
                        out=t2pre[:, cw].rearrange("p (b s) -> p b s", b=2),
                        in0=base2P[:, gsc].rearrange("p (b s) -> p b s", b=2),
                        in1=u2S[:, 2 * cb:2 * cb + 2, None]
                            .broadcast_to((H, 2, S)), op=OP.add)
                t2S = work.tile([H, GB * S], f32, tag=f"t2S{g}")
                nc.scalar.activation(t2S[:, 0:256], t2pre[:, 0:256], AFt)
                nc.scalar.activation(t2S[:, 256:512], t2pre[:, 256:512], AFt)
                for bl in range(GB):
                    nc.tensor.matmul(A2T[:, bl:bl + 1],
                                     t2S[:, bl * S:(bl + 1) * S], cs["vv2c"][:],
                                     start=True, stop=True)

                # ---- logits, argmax, bookkeeping ----
                logitsT = work.tile([S, GB], f32, tag=f"lgT{g}")
                nc.vector.tensor_tensor(out=logitsT[:], in0=A2T,
                                        in1=penaltyT[g][:], op=OP.add)
                nc.vector.tensor_copy(out=logbT[g][:, t * GB:(t + 1) * GB],
                                      in_=logitsT[:])
                nc.tensor.transpose(Lblk, logitsT[:], cs["I128"][:])
                LS = work.tile([GB, S], f32, tag=f"ls{g}")
                nc.vector.tensor_copy(out=LS[:], in_=Lblk)
                M8 = work.tile([GB, 8], f32, tag=f"m8{g}")
                nc.vector.max(M8[:], LS[:])
                I8u = work.tile([GB, 8], u32, tag=f"i8{g}")
                nc.vector.max_index(I8u[:], M8[:], LS[:])
                nc.vector.tensor_copy(out=ptrb[g][:, t:t + 1], in_=I8u[:, 0:1])
                OHb = work.tile([GB, S], f32, tag=f"oh{g}")
                nc.vector.tensor_scalar(out=OHb[:], in0=LS[:], scalar1=M8[:, 0:1],
                                        scalar2=None, op0=OP.is_equal)
                nc.tensor.transpose(OHT, OHb[:], cs["I128"][0:GB, 0:GB])
                ohT = work.tile([S, GB], f32, tag=f"ohT{g}")
                nc.vector.tensor_copy(out=ohT[:], in_=OHT)
                # next-step decoder input: dec_h = (W_dec @ static)[:, :, ptr]
                for bl in range(GB):
                    b = g * GB + bl
                    nc.tensor.matmul(DH[:, bl:bl + 1],
                                     WdecST[:, b * H:(b + 1) * H],
                                     ohT[:, bl:bl + 1], start=True, stop=True)
                nc.vector.tensor_copy(out=dec_hT[g][:], in_=DH)
                # penalty update (gpsimd, off critical path)
                tsp = work.tile([S, GB], f32, tag=f"tsp{g}")
                nc.gpsimd.tensor_scalar(out=tsp[:], in0=ohT[:], scalar1=NEG,
                                        scalar2=None, op0=OP.mult)
                nc.gpsimd.tensor_tensor(out=penaltyT[g][:], in0=penaltyT[g][:],
                                        in1=tsp[:], op=OP.add)

            for t in range(n_steps):
                for g in range(NG):
                    step(t, g)

            # ---- post-loop: logp = -ln(sum(exp(logits - max))) ----
            # logbT[g] is [s, (t,b)]; transpose 128-col chunks to [(t,b), s],
            # then exp(bias=-max) with fused row-sum, then ln, negate.
            nchunk = (GB * n_steps + S - 1) // S          # chunks per group
            sums = [state.tile([S, nchunk], f32, tag=f"sums{g}",
                               name=f"sums_{g}") for g in range(NG)]
            for g in range(NG):
                nc.vector.memset(sums[g][:], 1.0)
                for c in range(nchunk):
                    w0 = c * S
                    wid = min(S, GB * n_steps - w0)
                    pt = psB[g].tile([S, S], f32, tag="bkb", name=f"pT{g}{c}")
                    nc.tensor.transpose(pt[0:wid, :],
                                        logbT[g][:, w0:w0 + wid], cs["I128"][:])
                    blk = work.tile([S, S], f32, tag=f"pb{g}")
                    nc.vector.tensor_copy(out=blk[0:wid, :], in_=pt[0:wid, :])
                    nmx = work.tile([S, 1], f32, tag=f"nm{g}")
                    nc.vector.tensor_reduce(out=nmx[0:wid, :], in_=blk[0:wid, :],
                                            op=OP.max,
                                            axis=mybir.AxisListType.X,
                                            negate=True)
                    eb = work.tile([S, S], f32, tag=f"eb{g}")
                    nc.scalar.activation(eb[0:wid, :], blk[0:wid, :], AFe,
                                         bias=nmx[0:wid, :],
                                         accum_out=sums[g][0:wid, c:c + 1])
            logpb = [state.tile([S, nchunk], f32, tag=f"logpb{g}",
                                name=f"logpb_{g}") for g in range(NG)]
            for g in range(NG):
                lnb = work.tile([S, nchunk], f32, tag=f"lnb{g}")
                nc.scalar.activation(lnb[:], sums[g][:], AF.Ln)
                nc.vector.tensor_scalar(out=logpb[g][:], in0=lnb[:], scalar1=-1.0,
                                        scalar2=None, op0=OP.mult)
                nc.sync.dma_start(out_idx[g * GB:(g + 1) * GB, :], ptrb[g][:])
                nc.sync.dma_start(out_logp[:, g * nchunk:(g + 1) * nchunk],
                                  logpb[g][:])

    nc.compile()
    return nc


def host_inputs(static, dynamic, W_s, W_d, W_dec, vv1, ww1, vv2, ww2,
                W_ih, W_hh):
    """Per-core in_maps (layout transforms only; all heavy compute on-device)."""
    f = np.float32
    shared = {
        "WsT": np.ascontiguousarray(W_s.T, f),
        "WdT": np.ascontiguousarray(W_d.T, f),
        "ww1sT": np.ascontiguousarray(ww1[:, :H].T, f),
        "ww1dT": np.ascontiguousarray(ww1[:, H:2 * H].T, f),
        "w1hT": np.ascontiguousarray(ww1[:, 2 * H:].T, f),
        "ww2sT": np.ascontiguousarray(ww2[:, :H].T, f),
        "ww2dT": np.ascontiguousarray(ww2[:, 2 * H:].T, f),
        "w2dT": np.ascontiguousarray(ww2[:, H:2 * H].T, f),
        "WdecT": np.ascontiguousarray(W_dec.T, f),
        "WihT_r": np.ascontiguousarray(W_ih[:H].T, f),
        "WihT_z": np.ascontiguousarray(W_ih[H:2 * H].T, f),
        "WihT_n": np.ascontiguousarray(W_ih[2 * H:].T, f),
        "WhhT_r": np.ascontiguousarray(W_hh[:H].T, f),
        "WhhT_z": np.ascontiguousarray(W_hh[H:2 * H].T, f),
        "WhhT_nh": np.ascontiguousarray(0.5 * W_hh[2 * H:].T, f),
        "vv1c": np.ascontiguousarray(vv1[:, None], f),
        "vv2c": np.ascontiguousarray(vv2[:, None], f),
        "I128": np.eye(H, dtype=f),
        "ones128": np.ones((H, H), f),
    }
    in_maps = []
    for c in range(NCORES):
        bs = slice(c * BL, (c + 1) * BL)
        pen = np.where(dynamic[bs, 0, :] != 0, NEG, 0.0).astype(f)
        pen[:, 0] = NEG
        m = dict(shared)
        m["staticT8"] = np.ascontiguousarray(
            static[bs].transpose(1, 0, 2).reshape(SS, BL * S), f)
        m["dynT4"] = np.ascontiguousarray(
            dynamic[bs].transpose(1, 0, 2).reshape(DS, BL * S), f)
        m["penT0"] = np.ascontiguousarray(pen.T, f)
        in_maps.append(m)
    return in_maps


def unpack_outputs(results, n_steps=NSTEP):
    """results: list of 8 dicts with out_idx_raw/out_logp_raw."""
    nchunk = (GB * n_steps + S - 1) // S
    idxs, logps = [], []
    for res in results:
        idxs.append(res["out_idx_raw"].astype(np.int32))
        raw = res["out_logp_raw"]
        lp = np.zeros((BL, n_steps), np.float32)
        for g in range(NG):
            flat = raw[:, g * nchunk:(g + 1) * nchunk].T.reshape(-1)
            lp[g * GB:(g + 1) * GB, :] = \
                flat[:GB * n_steps].reshape(n_steps, GB).T
        logps.append(lp)
    return np.concatenate(idxs, 0), np.concatenate(logps, 0)


_CACHE = {}


def kernel(static, dynamic, transition_time, W_s, b_s, W_d, b_d, W_dec, b_dec,
           vv1, ww1, vv2, ww2, W_ih, W_hh, b_ih, b_hh):
    for bias in (b_s, b_d, b_dec, b_ih, b_hh):
        assert not np.any(np.asarray(bias)), "kernel assumes zero biases"
    from concourse.bass_utils import run_bass_kernel_spmd
    if "nc" not in _CACHE:
        _CACHE["nc"] = _build_nc()
    in_maps = host_inputs(np.asarray(static), np.asarray(dynamic),
                          np.asarray(W_s), np.asarray(W_d), np.asarray(W_dec),
                          np.asarray(vv1), np.asarray(ww1), np.asarray(vv2),
                          np.asarray(ww2), np.asarray(W_ih), np.asarray(W_hh))
    res = run_bass_kernel_spmd(_CACHE["nc"], in_maps,
                               core_ids=list(range(NCORES)))
    return unpack_outputs(res.results)



# revision 13
# speedup vs baseline: 1.2139x; 1.2139x over previous
"""Trainium2 Bass kernel for nn_DRL4SSP (pointer-network greedy decode).

Strategy: pure data-parallel over batch B=64 across 8 NeuronCores (8 items
per core). Inside each core the 127 sequential decode steps run fully
on-chip: encoders/bases are computed once in a prologue; the per-step
recurrence (GRU + two pointer-attention stages + greedy argmax) is executed
with all state resident in SBUF/PSUM. Two pipeline groups of 4 batch items
interleave phase-by-phase so every engine's in-order queue alternates
groups and one group's work fills the other's dependency stalls.

Layout / structural choices (per core, b = local batch 0..7, s = 0..127):
  base1P/base2P   [128(h), 1024(b-major, s)]  loop-invariant bias tensors
  MrT/MzT/MnT     [128(s), 1024(b-major, h)]  folded (W_ih_x @ W_dec @
                  static_b)^T so the GRU input gates are matmuls straight
                  from the previous one-hot: the whole decoder-conv stage
                  is eliminated from the recurrence.
  logits          stage-2 scores come out [s, b]; the penalty add writes
                  them STRAIGHT into the post-loop log store (no separate
                  store copy), one PE transpose gives the [b, s] block and
                  argmax/one-hot read it directly from PSUM.
  GRU elementwise uses scalar_tensor_tensor fusions (2 chain ops after
                  tanh(n)); softmax1 normalization is deferred past the U2
                  matmuls (u2 = (W2SH@e1)/sum1) so the matmuls overlap the
                  reciprocal.
All compute is fp32: bf16 was measured to flip 63/64 tours and float32r
(TF32-class) flipped 18/64, so exact-fp32 engines (DVE/ACT/Pool) do all
elementwise work. Log-prob denominators are deferred to a single post-loop
pass (keeps the hot loop on one ACT table set; exp..exp then ln ordering
pays exactly one table switch).
"""
import sys
import numpy as np

for _p in ("/opt/trn_rl_repo",):
    if _p not in sys.path:
        sys.path.insert(0, _p)

B, SS, DS, H, S = 64, 8, 4, 128, 128
NCORES = 8
BL = B // NCORES          # batch items per core = 8
NG = 2                    # pipeline groups per core
GB = BL // NG             # batch items per group = 4
NSTEP = S - 1             # 127
NEG = -1e30


def _build_nc(n_steps=NSTEP, bench_loop=1):
    from contextlib import ExitStack, nullcontext
    import concourse.bass as bass
    import concourse.tile as tile
    from concourse import bacc, mybir

    f32 = mybir.dt.float32
    u32 = mybir.dt.uint32
    AF = mybir.ActivationFunctionType
    OP = mybir.AluOpType

    nc = bacc.Bacc("TRN2", target_bir_lowering=False, debug=False,
                   enable_asserts=False)

    ncht = (GB * n_steps + S - 1) // S     # post-loop chunks per group

    # ---- DRAM I/O ----
    din = {}
    def dram_in(name, shape):
        din[name] = nc.dram_tensor(name, shape, f32, kind="ExternalInput").ap()
    dram_in("staticT8", [SS, BL * S])      # [i, (b,s)]
    dram_in("dynT4", [DS, BL * S])
    dram_in("penT0", [S, BL])              # penalty, transposed [s, b]
    for nm, shp in [("WsT", [SS, H]), ("WdT", [DS, H]),
                    ("ww1sT", [H, H]), ("ww1dT", [H, H]), ("w1hT", [H, H]),
                    ("ww2sT", [H, H]), ("ww2dT", [H, H]), ("w2dT", [H, H]),
                    ("WdecT", [SS, H]),
                    ("WihT_r", [H, H]), ("WihT_z", [H, H]), ("WihT_n", [H, H]),
                    ("WhhT_r", [H, H]), ("WhhT_z", [H, H]), ("WhhT_nh", [H, H]),
                    ("vv1c", [H, 1]), ("vv2c", [H, 1]),
                    ("I128", [H, H]), ("ones128", [H, H])]:
        dram_in(nm, shp)
    out_idx = nc.dram_tensor("out_idx_raw", [BL, n_steps], u32,
                             kind="ExternalOutput").ap()
    out_logp = nc.dram_tensor("out_logp_raw", [H, NG * ncht], f32,
                              kind="ExternalOutput").ap()

    with ExitStack() as ctx:
        tc = ctx.enter_context(tile.TileContext(nc))
        cpool = ctx.enter_context(tc.tile_pool(name="consts", bufs=1))
        state = ctx.enter_context(tc.tile_pool(name="state", bufs=1))
        work = ctx.enter_context(tc.tile_pool(name="work", bufs=3))

        loop_cm = tc.For_i(0, bench_loop, 1) if bench_loop > 1 else nullcontext()
        with loop_cm:
            # ---- load constants to SBUF ----
            # Every const is copied once through DVE so that downstream
            # matmuls depend on a single engine semaphore.
            cs = {}
            for nm, ap in din.items():
                raw = cpool.tile(list(ap.shape), f32, tag=f"r_{nm}", name=f"r_{nm}")
                nc.sync.dma_start(raw[:], ap[:])
                t = cpool.tile(list(ap.shape), f32, tag=nm, name=f"c_{nm}")
                nc.vector.tensor_copy(out=t[:], in_=raw[:])
                cs[nm] = t

            # ---- persistent state ----
            base1P = state.tile([H, BL * S], f32, tag="base1P")
            base2P = state.tile([H, BL * S], f32, tag="base2P")
            W2SHT = state.tile([S, BL * H], f32, tag="W2SHT")
            MrT = state.tile([S, BL * H], f32, tag="MrT")
            MzT = state.tile([S, BL * H], f32, tag="MzT")
            MnT = state.tile([S, BL * H], f32, tag="MnT")
            hT = [state.tile([H, GB], f32, tag=f"hT{g}", name=f"hT_{g}")
                  for g in range(NG)]
            ohT = [state.tile([S, GB], f32, tag=f"ohT{g}", name=f"ohT_{g}")
                   for g in range(NG)]
            penT = [state.tile([S, GB], f32, tag=f"penT{g}", name=f"penT_{g}")
                    for g in range(NG)]
            logbT = [state.tile([S, GB * n_steps], f32, tag=f"logbT{g}",
                                name=f"logbT_{g}") for g in range(NG)]
            ptrb = [state.tile([GB, n_steps], u32, tag=f"ptrb{g}",
                               name=f"ptrb_{g}") for g in range(NG)]
            shS = state.tile([H, BL * S], f32, tag="shS")       # static_h
            dhS = state.tile([H, BL * S], f32, tag="dhS")       # dynamic_h

            for g in range(NG):
                nc.vector.memset(hT[g][:], 0.0)
                nc.vector.memset(ohT[g][:], 0.0)
                nc.vector.memset(logbT[g][:], 0.0)
                nc.vector.tensor_copy(out=penT[g][:],
                                      in_=cs["penT0"][:, g * GB:(g + 1) * GB])

            # ---- prologue: encoders, bases, folded weights ----
            with tc.tile_pool(name="pro_ps", bufs=2, space="PSUM") as pps:
                def big_mm_to(dst, terms):
                    for half in range(2):
                        sl = slice(half * 512, half * 512 + 512)
                        pt = pps.tile([H, 512], f32, tag="pro")
                        for i, (lhsT, rhs) in enumerate(terms):
                            nc.tensor.matmul(pt[:], lhsT, rhs[:, sl],
                                             start=(i == 0),
                                             stop=(i == len(terms) - 1))
                        nc.vector.tensor_copy(out=dst[:, sl], in_=pt[:])

                big_mm_to(shS, [(cs["WsT"][:], cs["staticT8"][:])])
                big_mm_to(dhS, [(cs["WdT"][:], cs["dynT4"][:])])
                big_mm_to(base1P, [(cs["ww1sT"][:], shS[:]),
                                   (cs["ww1dT"][:], dhS[:])])
                big_mm_to(base2P, [(cs["ww2sT"][:], shS[:]),
                                   (cs["ww2dT"][:], dhS[:])])

                # W2SH = w2d @ static_h, then per-item transpose to [s, (b,h)]
                w2a = state.tile([H, BL * S], f32, tag="w2a")
                big_mm_to(w2a, [(cs["w2dT"][:], shS[:])])
                wda = state.tile([H, BL * S], f32, tag="wda")
                big_mm_to(wda, [(cs["WdecT"][:], cs["staticT8"][:])])
                for b in range(BL):
                    sl = slice(b * S, b * S + S)
                    pt = pps.tile([H, S], f32, tag="protr")
                    nc.tensor.transpose(pt[:], w2a[:, sl], cs["I128"][:])
                    nc.vector.tensor_copy(out=W2SHT[:, sl], in_=pt[:])
                # Folded GRU input gates: MxT_b = (W_ih_x @ W_dec @ static_b)^T
                #   = wda_b^T @ WihT_x, computed directly transposed.
                for (Mx, wn) in [(MrT, "WihT_r"), (MzT, "WihT_z"),
                                 (MnT, "WihT_n")]:
                    for half in range(2):
                        pt = pps.tile([S, 512], f32, tag="pro")
                        for q in range(4):
                            b = half * 4 + q
                            nc.tensor.matmul(pt[:, q * H:(q + 1) * H],
                                             wda[:, b * S:(b + 1) * S],
                                             cs[wn][:], start=True, stop=True)
                        nc.vector.tensor_copy(
                            out=Mx[:, half * 512:half * 512 + 512], in_=pt[:])

            # ---- main-loop PSUM pools (per group) ----
            psA = [ctx.enter_context(
                tc.tile_pool(name=f"Ag{g}", bufs=1, space="PSUM")) for g in range(NG)]
            psB = [ctx.enter_context(
                tc.tile_pool(name=f"Bg{g}", bufs=1, space="PSUM")) for g in range(NG)]
            # bankA: G_r 0:4 | G_z 4:8 | G_nacc 8:12 | G_h2 12:16 | U1 16:20
            #        | A1T 20:24 | S1rep 24:28 | U2raw 28:32
            bkA = [psA[g].tile([H, 512], f32, tag="bka", name=f"bkA_{g}")
                   for g in range(NG)]
            # bankB: A2T cols 0:4 | OHT cols 4:8 | Lb rows 0:4 cols 8:136
            bkB = [psB[g].tile([H, 512], f32, tag="bkb", name=f"bkB_{g}")
                   for g in range(NG)]

            AFt, AFe = AF.Tanh, AF.Exp

            # work tiles per group (allocated once per tag via pool reuse)
            def wt(name, shape, g, dt=f32):
                return work.tile(shape, dt, tag=f"{name}{g}",
                                 name=f"{name}_{g}")

            def ph_gates(t, g):
                ga = bkA[g]
                G_r, G_z = ga[:, 0:4], ga[:, 4:8]
                G_nacc, G_h2 = ga[:, 8:12], ga[:, 12:16]
                h_g, oh_g = hT[g][:], ohT[g][:]
                nc.tensor.matmul(G_r, cs["WhhT_r"][:], h_g, start=True, stop=False)
                nc.tensor.matmul(G_z, cs["WhhT_z"][:], h_g, start=True, stop=False)
                nc.tensor.matmul(G_nacc, cs["WhhT_nh"][:], h_g, start=True,
                                 stop=False)
                nc.tensor.matmul(G_h2, cs["WhhT_nh"][:], h_g, start=True,
                                 stop=True)
                for bl in range(GB):
                    b = g * GB + bl
                    bs = slice(b * H, (b + 1) * H)
                    nc.tensor.matmul(G_r[:, bl:bl + 1], MrT[:, bs],
                                     oh_g[:, bl:bl + 1], start=False,
                                     stop=(bl == GB - 1))
                    nc.tensor.matmul(G_z[:, bl:bl + 1], MzT[:, bs],
                                     oh_g[:, bl:bl + 1], start=False,
                                     stop=(bl == GB - 1))
                    nc.tensor.matmul(G_nacc[:, bl:bl + 1], MnT[:, bs],
                                     oh_g[:, bl:bl + 1], start=False,
                                     stop=(bl == GB - 1))

            def ph_trz(t, g):
                trz = wt("trz", [H, 2 * GB], g)
                nc.scalar.activation(trz[:], bkA[g][:, 0:8], AFt, scale=0.5)
                return trz

            def ph_small1(t, g, trz):
                # m = trz_r * G_h2 (chain); u = 1-z, ZH = 2*z*h (off-chain)
                m = wt("m", [H, GB], g)
                nc.vector.tensor_tensor(out=m[:], in0=trz[:, 0:GB],
                                        in1=bkA[g][:, 12:16], op=OP.mult)
                u = wt("u", [H, GB], g)
                nc.vector.tensor_scalar(out=u[:], in0=trz[:, GB:2 * GB],
                                        scalar1=-0.5, scalar2=0.5,
                                        op0=OP.mult, op1=OP.add)
                zh = wt("zh", [H, GB], g)
                nc.vector.scalar_tensor_tensor(out=zh[:], in0=trz[:, GB:2 * GB],
                                               scalar=1.0, in1=hT[g][:],
                                               op0=OP.add, op1=OP.mult)
                return m, u, zh

            def ph_nin(t, g, m):
                nin = wt("nin", [H, GB], g)
                nc.vector.tensor_tensor(out=nin[:], in0=m[:],
                                        in1=bkA[g][:, 8:12], op=OP.add)
                return nin

            def ph_tn(t, g, nin):
                tn = wt("tn", [H, GB], g)
                nc.scalar.activation(tn[:], nin[:], AFt)
                return tn

            def ph_h(t, g, tn, u, zh):
                # h' = tn*(1-z) + z*h = tn*u + 0.5*ZH
                p = wt("p", [H, GB], g)
                nc.vector.tensor_tensor(out=p[:], in0=tn[:], in1=u[:],
                                        op=OP.mult)
                nc.vector.scalar_tensor_tensor(out=hT[g][:], in0=zh[:],
                                               scalar=0.5, in1=p[:],
                                               op0=OP.mult, op1=OP.add)

            def ph_U1(t, g):
                nc.tensor.matmul(bkA[g][:, 16:20], cs["w1hT"][:], hT[g][:],
                                 start=True, stop=True)

            def ph_t1(t, g):
                t1pre = wt("t1p", [H, GB * S], g)
                U1 = bkA[g][:, 16:20]
                for cb in range(2):
                    cw = slice(cb * 256, cb * 256 + 256)
                    gsc = slice(g * GB * S + cb * 256,
                                g * GB * S + cb * 256 + 256)
                    nc.vector.tensor_tensor(
                        out=t1pre[:, cw].rearrange("p (b s) -> p b s", b=2),
                        in0=base1P[:, gsc].rearrange("p (b s) -> p b s", b=2),
                        in1=U1[:, 2 * cb:2 * cb + 2, None]
                            .broadcast_to((H, 2, S)), op=OP.add)
                return t1pre

            def ph_t1a(t, g, t1pre):
                t1S = wt("t1S", [H, GB * S], g)
                nc.scalar.activation(t1S[:, 0:256], t1pre[:, 0:256], AFt)
                nc.scalar.activation(t1S[:, 256:512], t1pre[:, 256:512], AFt)
                return t1S

            def ph_A1(t, g, t1S):
                for bl in range(GB):
                    nc.tensor.matmul(bkA[g][:, 20 + bl:21 + bl],
                                     t1S[:, bl * S:(bl + 1) * S], cs["vv1c"][:],
                                     start=True, stop=True)

            def ph_exp(t, g):
                e1T = wt("e1T", [S, GB], g)
                nc.scalar.activation(e1T[:], bkA[g][:, 20:24], AFe)
                return e1T

            def ph_S1U2(t, g, e1T):
                nc.tensor.matmul(bkA[g][:, 24:28], cs["ones128"][:], e1T[:],
                                 start=True, stop=True)
                for bl in range(GB):
                    b = g * GB + bl
                    nc.tensor.matmul(bkA[g][:, 28 + bl:29 + bl],
                                     W2SHT[:, b * H:(b + 1) * H],
                                     e1T[:, bl:bl + 1], start=True, stop=True)

            def ph_recip(t, g):
                r1 = wt("r1", [S, GB], g)
                nc.vector.reciprocal(r1[:], bkA[g][:, 24:28])
                return r1

            def ph_u2(t, g, r1):
                u2S = wt("u2S", [H, GB], g)
                nc.vector.tensor_tensor(out=u2S[:], in0=bkA[g][:, 28:32],
                                        in1=r1[:], op=OP.mult)
                return u2S

            def ph_t2(t, g, u2S):
                t2pre = wt("t2p", [H, GB * S], g)
                for cb in range(2):
                    cw = slice(cb * 256, cb * 256 + 256)
                    gsc = slice(g * GB * S + cb * 256,
                                g * GB * S + cb * 256 + 256)
                    nc.vector.tensor_tensor(
                        out=t2pre[:, cw].rearrange("p (b s) -> p b s", b=2),
                        in0=base2P[:, gsc].rearrange("p (b s) -> p b s", b=2),
                        in1=u2S[:, 2 * cb:2 * cb + 2, None]
                            .broadcast_to((H, 2, S)), op=OP.add)
                return t2pre

            def ph_t2a(t, g, t2pre):
                t2S = wt("t2S", [H, GB * S], g)
                nc.scalar.activation(t2S[:, 0:256], t2pre[:, 0:256], AFt)
                nc.scalar.activation(t2S[:, 256:512], t2pre[:, 256:512], AFt)
                return t2S

            def ph_A2(t, g, t2S):
                for bl in range(GB):
                    nc.tensor.matmul(bkB[g][:, bl:bl + 1], t2S[:, bl * S:(bl + 1) * S],
                                     cs["vv2c"][:], start=True, stop=True)

            def ph_logit(t, g):
                # logits = A2T + pen, written straight into the post-loop
                # log store (this IS the store; [S, GB] so the add is cheap)
                nc.vector.tensor_tensor(
                    out=logbT[g][:, t * GB:(t + 1) * GB], in0=bkB[g][:, 0:4],
                    in1=penT[g][:], op=OP.add)

            def ph_tr1(t, g):
                nc.tensor.transpose(bkB[g][0:GB, 8:136],
                                    logbT[g][:, t * GB:(t + 1) * GB],
                                    cs["I128"][:])

            def ph_max(t, g):
                M8 = wt("m8", [GB, 8], g)
                nc.vector.max(M8[:], bkB[g][0:GB, 8:136])
                return M8

            def ph_oh(t, g, M8):
                OH = wt("oh", [GB, S], g)
                nc.vector.tensor_scalar(out=OH[:], in0=bkB[g][0:GB, 8:136],
                                        scalar1=M8[:, 0:1], scalar2=None,
                                        op0=OP.is_equal)
                return OH

            def ph_tr2(t, g, OH):
                nc.tensor.transpose(bkB[g][:, 4:8], OH[:],
                                    cs["I128"][0:GB, 0:GB])

            def ph_ohc(t, g):
                nc.vector.tensor_copy(out=ohT[g][:], in_=bkB[g][:, 4:8])

            def ph_off(t, g, M8):
                # bookkeeping off the recurrence critical path
                I8u = wt("i8", [GB, 8], g, u32)
                nc.vector.max_index(I8u[:], M8[:], bkB[g][0:GB, 8:136])
                nc.vector.tensor_copy(out=ptrb[g][:, t:t + 1], in_=I8u[:, 0:1])
                tsp = wt("tsp", [S, GB], g)
                nc.gpsimd.tensor_scalar(out=tsp[:], in0=ohT[g][:], scalar1=NEG,
                                        scalar2=None, op0=OP.mult)
                nc.gpsimd.tensor_tensor(out=penT[g][:], in0=penT[g][:],
                                        in1=tsp[:], op=OP.add)

            # ---- decode loop: fine-grained group interleave ----
            for t in range(n_steps):
                ctxg = [{} for _ in range(NG)]
                for g in range(NG):
                    ph_gates(t, g)
                for g in range(NG):
                    ctxg[g]["trz"] = ph_trz(t, g)
                for g in range(NG):
                    ctxg[g]["m"], ctxg[g]["u"], ctxg[g]["zh"] = \
                        ph_small1(t, g, ctxg[g]["trz"])
                for g in range(NG):
                    ctxg[g]["nin"] = ph_nin(t, g, ctxg[g]["m"])
                for g in range(NG):
                    ctxg[g]["tn"] = ph_tn(t, g, ctxg[g]["nin"])
                for g in range(NG):
                    ph_h(t, g, ctxg[g]["tn"], ctxg[g]["u"], ctxg[g]["zh"])
                for g in range(NG):
                    ph_U1(t, g)
                for g in range(NG):
                    ctxg[g]["t1pre"] = ph_t1(t, g)
                for g in range(NG):
                    ctxg[g]["t1S"] = ph_t1a(t, g, ctxg[g]["t1pre"])
                for g in range(NG):
                    ph_A1(t, g, ctxg[g]["t1S"])
                for g in range(NG):
                    ctxg[g]["e1T"] = ph_exp(t, g)
                for g in range(NG):
                    ph_S1U2(t, g, ctxg[g]["e1T"])
                for g in range(NG):
                    ctxg[g]["r1"] = ph_recip(t, g)
                for g in range(NG):
                    ctxg[g]["u2S"] = ph_u2(t, g, ctxg[g]["r1"])
                for g in range(NG):
                    ctxg[g]["t2pre"] = ph_t2(t, g, ctxg[g]["u2S"])
                for g in range(NG):
                    ctxg[g]["t2S"] = ph_t2a(t, g, ctxg[g]["t2pre"])
                for g in range(NG):
                    ph_A2(t, g, ctxg[g]["t2S"])
                for g in range(NG):
                    ph_logit(t, g)
                for g in range(NG):
                    ph_tr1(t, g)
                for g in range(NG):
                    ctxg[g]["M8"] = ph_max(t, g)
                for g in range(NG):
                    ctxg[g]["OH"] = ph_oh(t, g, ctxg[g]["M8"])
                for g in range(NG):
                    ph_tr2(t, g, ctxg[g]["OH"])
                for g in range(NG):
                    ph_ohc(t, g)
                for g in range(NG):
                    ph_off(t, g, ctxg[g]["M8"])

            # ---- post-loop: logp = -ln(sum(exp(logits - max))) ----
            # logbT[g] is [s, (t,b)]; transpose 128-col chunks to [(t,b), s],
            # then exp(bias=-max) with fused row-sum. All exps (both groups)
            # run before the Lns so the ACT table switches exactly once.
            sums = [state.tile([S, ncht], f32, tag=f"sums{g}",
                               name=f"sums_{g}") for g in range(NG)]
            for g in range(NG):
                nc.vector.memset(sums[g][:], 1.0)
            for g in range(NG):
                for c in range(ncht):
                    w0 = c * S
                    wid = min(S, GB * n_steps - w0)
                    pt = psB[g].tile([S, S], f32, tag="bkb", name=f"pT{g}{c}")
                    nc.tensor.transpose(pt[0:wid, :],
                                        logbT[g][:, w0:w0 + wid], cs["I128"][:])
                    blk = wt("pb", [S, S], g)
                    nc.vector.tensor_copy(out=blk[0:wid, :], in_=pt[0:wid, :])
                    nmx = wt("nm", [S, 1], g)
                    nc.vector.tensor_reduce(out=nmx[0:wid, :], in_=blk[0:wid, :],
                                            op=OP.max,
                                            axis=mybir.AxisListType.X,
                                            negate=True)
                    eb = wt("eb", [S, S], g)
                    nc.scalar.activation(eb[0:wid, :], blk[0:wid, :], AFe,
                                         bias=nmx[0:wid, :],
                                         accum_out=sums[g][0:wid, c:c + 1])
            logpb = [state.tile([S, ncht], f32, tag=f"logpb{g}",
                                name=f"logpb_{g}") for g in range(NG)]
            for g in range(NG):
                lnb = wt("lnb", [S, ncht], g)
                nc.scalar.activation(lnb[:], sums[g][:], AF.Ln)
                nc.vector.tensor_scalar(out=logpb[g][:], in0=lnb[:],
                                        scalar1=-1.0, scalar2=None,
                                        op0=OP.mult)
                nc.sync.dma_start(out_idx[g * GB:(g + 1) * GB, :], ptrb[g][:])
                nc.sync.dma_start(out_logp[:, g * ncht:(g + 1) * ncht],
                                  logpb[g][:])

    nc.compile()
    return nc


def host_inputs(static, dynamic, W_s, W_d, W_dec, vv1, ww1, vv2, ww2,
                W_ih, W_hh):
    """Per-core in_maps (layout transforms only; all heavy compute on-device)."""
    f = np.float32
    shared = {
        "WsT": np.ascontiguousarray(W_s.T, f),
        "WdT": np.ascontiguousarray(W_d.T, f),
        "ww1sT": np.ascontiguousarray(ww1[:, :H].T, f),
        "ww1dT": np.ascontiguousarray(ww1[:, H:2 * H].T, f),
        "w1hT": np.ascontiguousarray(ww1[:, 2 * H:].T, f),
        "ww2sT": np.ascontiguousarray(ww2[:, :H].T, f),
        "ww2dT": np.ascontiguousarray(ww2[:, 2 * H:].T, f),
        "w2dT": np.ascontiguousarray(ww2[:, H:2 * H].T, f),
        "WdecT": np.ascontiguousarray(W_dec.T, f),
        "WihT_r": np.ascontiguousarray(W_ih[:H].T, f),
        "WihT_z": np.ascontiguousarray(W_ih[H:2 * H].T, f),
        "WihT_n": np.ascontiguousarray(W_ih[2 * H:].T, f),
        "WhhT_r": np.ascontiguousarray(W_hh[:H].T, f),
        "WhhT_z": np.ascontiguousarray(W_hh[H:2 * H].T, f),
        "WhhT_nh": np.ascontiguousarray(0.5 * W_hh[2 * H:].T, f),
        "vv1c": np.ascontiguousarray(vv1[:, None], f),
        "vv2c": np.ascontiguousarray(vv2[:, None], f),
        "I128": np.eye(H, dtype=f),
        "ones128": np.ones((H, H), f),
    }
    in_maps = []
    for c in range(NCORES):
        bs = slice(c * BL, (c + 1) * BL)
        pen = np.where(dynamic[bs, 0, :] != 0, NEG, 0.0).astype(f)
        pen[:, 0] = NEG
        m = dict(shared)
        m["staticT8"] = np.ascontiguousarray(
            static[bs].transpose(1, 0, 2).reshape(SS, BL * S), f)
        m["dynT4"] = np.ascontiguousarray(
            dynamic[bs].transpose(1, 0, 2).reshape(DS, BL * S), f)
        m["penT0"] = np.ascontiguousarray(pen.T, f)
        in_maps.append(m)
    return in_maps


def unpack_outputs(results, n_steps=NSTEP):
    """results: list of 8 dicts with out_idx_raw/out_logp_raw."""
    ncht = (GB * n_steps + S - 1) // S
    idxs, logps = [], []
    for res in results:
        idxs.append(res["out_idx_raw"].astype(np.int32))
        raw = res["out_logp_raw"]
        lp = np.zeros((BL, n_steps), np.float32)
        for g in range(NG):
            flat = raw[:, g * ncht:(g + 1) * ncht].T.reshape(-1)
            lp[g * GB:(g + 1) * GB, :] = \
                flat[:GB * n_steps].reshape(n_steps, GB).T
        logps.append(lp)
    return np.concatenate(idxs, 0), np.concatenate(logps, 0)


_CACHE = {}


def kernel(static, dynamic, transition_time, W_s, b_s, W_d, b_d, W_dec, b_dec,
           vv1, ww1, vv2, ww2, W_ih, W_hh, b_ih, b_hh):
    for bias in (b_s, b_d, b_dec, b_ih, b_hh):
        assert not np.any(np.asarray(bias)), "kernel assumes zero biases"
    from concourse.bass_utils import run_bass_kernel_spmd
    if "nc" not in _CACHE:
        _CACHE["nc"] = _build_nc()
    in_maps = host_inputs(np.asarray(static), np.asarray(dynamic),
                          np.asarray(W_s), np.asarray(W_d), np.asarray(W_dec),
                          np.asarray(vv1), np.asarray(ww1), np.asarray(vv2),
                          np.asarray(ww2), np.asarray(W_ih), np.asarray(W_hh))
    res = run_bass_kernel_spmd(_CACHE["nc"], in_maps,
                               core_ids=list(range(NCORES)))
    return unpack_outputs(res.results)
